# revision 23
# baseline (speedup 1.0000x reference)
"""Trainium2 Bass kernel for nn_ActorNetwork (GNN message passing), 8 NeuronCores.

Strategy
--------
Data-parallel over the 256 graphs: core c owns graphs [32c, 32c+32).

Algebraic restructure (validated vs reference to ~7e-7 rel err in f32,
~5.7e-3 with fp8/bf16 quantization):
  * GCNConv aggregation is a dense per-graph matmul with the block-diagonal
    normalized adjacency built on host from the edge list.
  * p-encoder only feeds its mean-pooled graph embedding forward:
      p_graph = pW2^T (sum_n relu(q[:, n])) + pb2,  q = W01^T (x_aug^T A_c)
    The relu-sum uses sum relu(x) = (sum x + sum |x|)/2: the linear part
    comes free as an extra adjacency column (row-sums), the |x| part is one
    DVE abs-reduce per graph straight out of PSUM - no relu materialization.
  * The layer-1 bias term pb1*c'[n] rides through the adjacency matmul as an
    extra source row (row 500 = c'), paired with an indicator column in x_aug.
  * mm1 (x_aug^T A_c) runs as fp8 DoubleRow matmuls (K=256/instr, 0.5
    cycles/row).  All 4 graphs of a wave share one K=2048 accumulation with a
    block-diagonal stationary (graph g at stationary cols [32g, 32g+18)),
    so the whole wave lands in ONE [128, 504] PSUM bank -> one cast drains 4
    graphs.  The stationary zero-padding is materialized on-chip (gpsimd
    memset once + small expansion copies), not shipped over HBM.
  * v-encoder: vb1 bias rides through A_v via a fake pad-node row; per-pair
    PSUM tiles are stacked 4-to-a-bank (col tile positions / free slots with
    a shared K-base per bank) so drains are [128, 512].
  * Head folds: h2/h0 are never materialized; hW1 blocks are pre-multiplied
    into W_a = vW2@hW1[0:128] (consumes av1), W_b = [vW0;vb0]@hW1[128:256]
    (consumes vxTa), W_c = vW2@hW1[256:384]/NV (consumes sum(av1)),
    W_d = pW2@hW1[384:512]*0.5 (consumes Z); all constant bias terms fold
    into hb1c.  The head runs in 4 quarters pipelined with the p-waves.
"""

import os
import numpy as np
from ml_dtypes import bfloat16, float8_e4m3

B, NP, NV, E = 256, 500, 50, 128
NC = 8
GPC = B // NC          # 32 graphs per core
NVP = 64               # padded v nodes per graph
VN = GPC * NVP         # 2048 padded v nodes per core
WAVES = 8
GPW = GPC // WAVES     # 4 graphs per wave
NCOLP = 504            # 500 dst cols + 1 rowsum col + 3 pad
PSCALE = 128.0         # fp8 scale folded into w01x


def _mk_bspec():
    bf = {}
    off = 0
    for name, P, F in [("avt", 128, 2048), ("vxt", 128, 16 * 18),
                       ("w01v4", 128, 128), ("w01x4", 128, 128),
                       ("W_a", 128, 256), ("W_c", 128, 256),
                       ("W_d", 128, 256), ("hw2c", 128, 256),
                       ("hw3", 128, 1),
                       ("vxTa", 17, VN), ("W_b", 17, 256),
                       ("gexp", 8, VN)]:
        bf[name] = (P, F, off)
        off += F
    return bf, off


BSPEC, BCOLS = _mk_bspec()

LAST_RESULTS = None
_nc_cache = None


def _build_nc():
    import concourse.bass as bass
    import concourse.bacc as bacc
    import concourse.mybir as mybir
    from concourse.tile import TileContext

    dt = mybir.dt
    f32, bf16 = dt.float32, dt.bfloat16
    AF = mybir.ActivationFunctionType
    AX = mybir.AxisListType
    OP = mybir.AluOpType
    DR = mybir.MatmulPerfMode.DoubleRow

    nc = bacc.Bacc("TRN2", target_bir_lowering=False, debug=False)

    pa_d = nc.declare_dram_parameter("pa", [WAVES, 128, 8 * 2 * NCOLP],
                                     dt.float8e4, isOutput=False)
    px_d = nc.declare_dram_parameter("px", [128, WAVES * 2 * 2 * GPW * 18],
                                     dt.float8e4, isOutput=False)
    bb_d = nc.declare_dram_parameter("bblob", [128, BCOLS], bf16, isOutput=False)
    fb_d = nc.declare_dram_parameter("fblob", [128, 4], f32, isOutput=False)
    out_d = nc.declare_dram_parameter("out", [1, GPC * NV], f32, isOutput=True)

    with TileContext(nc) as tc:
        with (
            tc.tile_pool(name="const", bufs=1) as cp,
            tc.tile_pool(name="pa", bufs=4) as pap,
            tc.tile_pool(name="ya", bufs=3) as yap,
            tc.tile_pool(name="big", bufs=1) as bp,
            tc.tile_pool(name="psA", bufs=2, space="PSUM") as psA,
            tc.tile_pool(name="psB", bufs=3, space="PSUM") as psB,
            tc.tile_pool(name="psC", bufs=2, space="PSUM") as psC,
            tc.tile_pool(name="psD", bufs=1, space="PSUM") as psD,
        ):
            bb = cp.tile([128, BCOLS], bf16, tag="bb", name="bb")
            for name in BSPEC:
                P, F, off = BSPEC[name]
                nc.sync.dma_start(out=bb[0:P, off:off + F],
                                  in_=bb_d[0:P, off:off + F])
            fb = cp.tile([128, 4], f32, tag="fb", name="fb")
            nc.sync.dma_start(out=fb[:], in_=fb_d[:])
            pxs = cp.tile([128, WAVES * 2 * 2 * GPW * 18], dt.float8e4,
                          tag="pxs", name="pxs")
            nc.sync.dma_start(out=pxs[:], in_=px_d[:])

            def bslc(name):
                P, F, off = BSPEC[name]
                return bb[0:P, off:off + F]

            avt = bslc("avt")
            vxt = bslc("vxt")
            vxTa = bslc("vxTa")
            gexp = bslc("gexp")
            w01x4 = bslc("w01x4")
            w01v4 = bslc("w01v4")
            W_a = bslc("W_a")
            W_b = bslc("W_b")
            W_c = bslc("W_c")
            W_d = bslc("W_d")
            hw2c = bslc("hw2c")
            hw3 = bslc("hw3")

            # persistent SBUF tiles
            px_ts = [cp.tile([128, 2176], dt.float8e4, tag=f"pxb{i}",
                             name=f"pxb{i}") for i in range(4)]
            for i in range(4):
                nc.scalar.memzero(px_ts[i][:])
            warm = bp.tile([1, 4], f32, tag="warm")
            nc.scalar.activation(out=warm[:], in_=fb[0:1, 0:4],
                                 func=AF.Lrelu, alpha=0.01)
            nc.scalar.activation(out=warm[:], in_=warm[:], func=AF.Abs)
            yav_t = bp.tile([128, 512], bf16, tag="yav")
            h1_t = bp.tile([128, VN], bf16, tag="h1")
            av1_t = bp.tile([128, VN], bf16, tag="av1")
            Sav_f = bp.tile([128, GPC], f32, tag="Savf")
            Sav_t = bp.tile([128, GPC], bf16, tag="Sav")
            Za = bp.tile([128, GPC], f32, tag="Za")    # DVE-written
            Zb = bp.tile([128, GPC], f32, tag="Zb")    # Act-written
            Zs = bp.tile([128, GPC], f32, tag="Zs")    # Act-written
            Zs2 = bp.tile([128, GPC], f32, tag="Zs2")  # DVE-written
            Z_t = bp.tile([128, GPC], bf16, tag="Z")
            scr = bp.tile([128, 504], bf16, tag="scr")
            gg_t = bp.tile([8, 256], bf16, tag="gg")
            xh_ts = [bp.tile([128, GPC * NV], bf16, tag=f"xh{b}", name=f"xh{b}")
                     for b in range(2)]
            hm_t = bp.tile([128, GPC * NV], bf16, tag="hm")
            ob = bp.tile([1, GPC * NV], f32, tag="ob")

            # ---------------- v encoder (emitted interleaved with waves) --
            def v_yav():
                yav_ps = psC.tile([128, 512], f32, tag="mC", name="yav_ps")
                for j in range(16):
                    nc.tensor.matmul(
                        out=yav_ps[32 * (j % 4):32 * (j % 4) + 18,
                                   128 * (j // 4):128 * (j // 4) + 128],
                        lhsT=vxt[:, 18 * j:18 * j + 18],
                        rhs=avt[:, 128 * j:128 * j + 128],
                        start=True, stop=True,
                        tile_position=(0, 32 * (j % 4)),
                    )
                nc.vector.tensor_copy(out=yav_t[:], in_=yav_ps[:])

            def v_qv():
                h1v = h1_t[:].rearrange("p (c f q) -> p c f q", c=4, f=4)
                for r in range(4):
                    a = 32 * r
                    qv_ps = psC.tile([128, 512], f32, tag="mC", name="qv_ps")
                    for c in range(4):
                        nc.tensor.matmul(
                            out=qv_ps[:, 128 * c:128 * c + 128],
                            lhsT=yav_t[a:a + 18, 128 * c:128 * c + 128],
                            rhs=w01v4[a:a + 18, :],
                            start=True, stop=True,
                            tile_position=(a, 0),
                        )
                    nc.scalar.activation(out=h1v[:, :, r, :], in_=qv_ps[:],
                                         func=AF.Relu)

            def v_av1():
                for jj in range(4):
                    av_ps = psC.tile([128, 512], f32, tag="mC", name="av_ps")
                    for j2 in range(4):
                        j = jj * 4 + j2
                        nc.tensor.matmul(
                            out=av_ps[:, 128 * j2:128 * j2 + 128],
                            lhsT=h1_t[:, 128 * j:128 * j + 128],
                            rhs=avt[:, 128 * j:128 * j + 128],
                            start=True, stop=True,
                        )
                    nc.vector.tensor_copy(
                        out=av1_t[:, 512 * jj:512 * jj + 512], in_=av_ps[:])

            def v_sav():
                nc.vector.tensor_reduce(
                    out=Sav_f[:],
                    in_=av1_t[:].rearrange("p (g n) -> p g n",
                                           n=NVP)[:, :, 0:NV],
                    axis=AX.X, op=OP.add,
                )
                nc.vector.tensor_copy(out=Sav_t[:], in_=Sav_f[:])

            # ---------------- p waves + pipelined head quarters ----------
            def mm1(wv):
                pa_t = pap.tile([128, 8 * 2 * NCOLP], dt.float8e4,
                                tag="pa", name="pa_t")
                H = 4 * 2 * NCOLP
                nc.gpsimd.dma_start(out=pa_t[:, 0:H], in_=pa_d[wv][:, 0:H])
                nc.gpsimd.dma_start(out=pa_t[:, H:2 * H],
                                    in_=pa_d[wv][:, H:2 * H])
                # expand this wave's x_aug blocks into the zero-padded
                # block-diagonal stationary buffer (4 strided copies)
                pxb = px_ts[wv % 4]
                for i in range(2):
                    for pp in range(2):
                        soff = ((wv * 2 + i) * 2 + pp) * GPW * 18
                        nc.gpsimd.tensor_copy(
                            out=pxb[:, 1024 * i + 512 * pp:
                                    1024 * i + 512 * pp + 640].rearrange(
                                "p (g r) -> p g r", g=4)[:, :, 0:18],
                            in_=pxs[:, soff:soff + GPW * 18].rearrange(
                                "p (g j) -> p g j", g=4),
                        )
                pxv = pxb[:, 0:2048].rearrange("p (i q) -> p i q", i=2)
                yps = psA.tile([128, NCOLP], f32, tag="mA", name="yps")
                for kp in range(8):
                    nc.tensor.matmul(
                        out=yps[:],
                        lhsT=pxv[:, :, 128 * kp:128 * kp + 128],
                        rhs=pa_t[:, 2 * NCOLP * kp:2 * NCOLP * (kp + 1)]
                            .rearrange("p (i c) -> p i c", i=2),
                        start=(kp == 0), stop=(kp == 7),
                        perf_mode=DR,
                    )
                ya_t = yap.tile([128, NCOLP], bf16, tag="ya", name="ya_t")
                nc.vector.tensor_copy(out=ya_t[:], in_=yps[:])
                return ya_t

            def mm2(wv, ya_t):
                for gj in range(GPW):
                    g = wv * GPW + gj
                    a = 32 * gj
                    qps = psB.tile([128, NCOLP], f32, tag="mB", name="qps")
                    nc.tensor.matmul(
                        out=qps[:],
                        lhsT=w01x4[a:a + 18, :],
                        rhs=ya_t[a:a + 18, :],
                        start=True, stop=True,
                        tile_position=(a, 0),
                    )
                    if gj == 3 and wv % 2 == 1:
                        nc.scalar.activation(out=scr[:, 0:500],
                                             in_=qps[:, 0:500], func=AF.Abs,
                                             accum_out=Zb[:, g:g + 1])
                        nc.vector.tensor_copy(out=Zs2[:, g:g + 1],
                                              in_=qps[:, 500:501])
                    else:
                        nc.vector.tensor_reduce(
                            out=Za[:, g:g + 1], in_=qps[:, 0:500],
                            axis=AX.X, op=OP.add, apply_absolute_value=True,
                        )
                        nc.scalar.activation(out=Zs[:, g:g + 1],
                                             in_=qps[:, 500:501], func=AF.Copy)

            def head_pre(Q):
                Sv = slice(512 * Q, 512 * Q + 512)
                banks = []
                for blk in range(2):
                    bs = slice(128 * blk, 128 * blk + 128)
                    xh_ps = psC.tile([128, 512], f32, tag="mC", name="xh_ps")
                    nc.tensor.matmul(out=xh_ps[:], lhsT=W_a[:, bs],
                                     rhs=av1_t[:, Sv], start=True, stop=False)
                    nc.tensor.matmul(out=xh_ps[:], lhsT=W_b[:, bs],
                                     rhs=vxTa[:, Sv], start=False, stop=False)
                    banks.append(xh_ps)
                return banks

            def head_quarter(Q, banks):
                gs = slice(8 * Q, 8 * Q + 8)
                S = slice(400 * Q, 400 * Q + 400)
                Ztv = Z_t[:].rearrange("p (w j) -> p w j", j=4)
                Zav = Za[:].rearrange("p (w j) -> p w j", j=4)
                Zsv = Zs[:].rearrange("p (w j) -> p w j", j=4)
                Wv = slice(2 * Q, 2 * Q + 2)
                We = slice(2 * Q, 2 * Q + 1)
                Wo = slice(2 * Q + 1, 2 * Q + 2)
                nc.vector.tensor_tensor(
                    out=Ztv[:, Wv, 0:3], in0=Zav[:, Wv, 0:3],
                    in1=Zsv[:, Wv, 0:3], op=OP.add)
                nc.vector.tensor_tensor(
                    out=Ztv[:, We, 3:4], in0=Zav[:, We, 3:4],
                    in1=Zsv[:, We, 3:4], op=OP.add)
                nc.vector.tensor_tensor(
                    out=Ztv[:, Wo, 3:4],
                    in0=Zb[:].rearrange("p (w j) -> p w j", j=4)[:, Wo, 3:4],
                    in1=Zs2[:].rearrange("p (w j) -> p w j", j=4)[:, Wo, 3:4],
                    op=OP.add)
                gg_ps = psD.tile([8, 256], f32, tag="mD", name="gg_ps")
                nc.tensor.matmul(out=gg_ps[:], lhsT=Sav_t[:, gs], rhs=W_c[:],
                                 start=True, stop=False)
                nc.tensor.matmul(out=gg_ps[:], lhsT=Z_t[:, gs], rhs=W_d[:],
                                 start=False, stop=True)
                nc.vector.tensor_copy(out=gg_t[:], in_=gg_ps[:])
                Sv = slice(512 * Q, 512 * Q + 512)
                for blk in range(2):
                    bs = slice(128 * blk, 128 * blk + 128)
                    xh_ps = banks[blk]
                    nc.tensor.matmul(out=xh_ps[:], lhsT=gg_t[:, bs],
                                     rhs=gexp[:, Sv],
                                     start=False, stop=True)
                    nc.scalar.activation(
                        out=xh_ts[blk][:, S],
                        in_=xh_ps[:].rearrange("p (g n) -> p g n", n=NVP)[:, :, 0:NV],
                        func=AF.Lrelu,
                        bias=fb[:, blk:blk + 1], alpha=0.01)
                hm_ps = psC.tile([128, 400], f32, tag="mC", name="hm_ps")
                nc.tensor.matmul(out=hm_ps[:], lhsT=hw2c[:, 0:128],
                                 rhs=xh_ts[0][:, S], start=True, stop=False)
                nc.tensor.matmul(out=hm_ps[:], lhsT=hw2c[:, 128:256],
                                 rhs=xh_ts[1][:, S], start=False, stop=True)
                nc.scalar.activation(out=hm_t[:, S], in_=hm_ps[:],
                                     func=AF.Lrelu, bias=fb[:, 2:3],
                                     alpha=0.01)
                ob_ps = psD.tile([1, 400], f32, tag="mD", name="ob_ps")
                nc.tensor.matmul(out=ob_ps[:], lhsT=hw3[:], rhs=hm_t[:, S],
                                 start=True, stop=True)
                nc.scalar.activation(out=ob[:, S], in_=ob_ps[:],
                                     func=AF.Identity, bias=fb[0:1, 3:4])
                nc.sync.dma_start(out=out_d[:, S], in_=ob[:, S])

            # software pipeline: emit mm2(wv-1) before mm1(wv) so the PE has
            # ready work queued while wave wv's pa DMA is still streaming;
            # head quarters fire as their graphs complete.
            v_yav()
            ya0 = mm1(0)
            v_qv()
            ya1 = mm1(1)
            v_av1()
            mm2(0, ya0)
            v_sav()
            hbanks = head_pre(0)
            ya2 = mm1(2)
            mm2(1, ya1)
            head_quarter(0, hbanks)
            yas = {2: ya2}
            for wv in range(3, WAVES):
                mm2(wv - 1, yas.pop(wv - 1))
                if (wv - 1) % 2 == 0:
                    hbanks = head_pre((wv - 1) // 2)
                else:
                    head_quarter((wv - 1) // 2, hbanks)
                yas[wv] = mm1(wv)
            mm2(WAVES - 1, yas.pop(WAVES - 1))
            head_quarter(3, hbanks)

    nc.compile()
    return nc


def _host_prep(inp):
    f32 = np.float32
    px = np.asarray(inp["p_x"], f32)
    vx = np.asarray(inp["v_x"], f32)
    pei = np.asarray(inp["p_edge_index"]).astype(np.int64)
    vei = np.asarray(inp["v_edge_index"]).astype(np.int64)
    g = {k: np.asarray(inp[k], f32) for k in
         ("pW0", "pb0", "pW1", "pb1", "pW2", "pb2",
          "vW0", "vb0", "vW1", "vb1", "vW2", "vb2",
          "hW1", "hb1", "hW2", "hb2", "hW3", "hb3")}

    # ---- p-side adjacency ----
    psrc, pdst = pei[0], pei[1]
    pdeg = 1.0 + np.bincount(pdst, minlength=B * NP).astype(f32)
    pdinv = (1.0 / np.sqrt(pdeg)).astype(f32)
    csum = pdinv * np.bincount(psrc, weights=pdinv[pdst],
                               minlength=B * NP).astype(f32)
    cp = (csum + pdinv * pdinv) / NP
    pa = np.zeros((B, 512, NCOLP), f32)
    w = (pdinv[psrc] * pdinv[pdst] * cp[pdst]).astype(f32)
    np.add.at(pa, (pdst // NP, psrc % NP, pdst % NP), w)
    ar = np.arange(B * NP)
    pa[ar // NP, ar % NP, ar % NP] += pdinv * pdinv * cp
    pa[:, 500, :500] = cp.reshape(B, NP)            # pb1 carrier row
    pa[:, :, 500] = pa[:, :, :500].sum(axis=2)      # rowsum col (Zs)
    # [B, 512, 504] -> [core, wave, p, kp'=pp*4+g, i, c]
    pa8 = (np.ascontiguousarray(
        (pa * PSCALE).reshape(NC, WAVES, GPW, 2, 2, 128, NCOLP)
        .transpose(0, 1, 5, 3, 2, 4, 6)
    ).reshape(NC, WAVES, 128, 8 * 2 * NCOLP)).astype(float8_e4m3)

    pxa = np.zeros((B, 512, 18), f32)
    pxa[:, :NP, :16] = px.reshape(B, NP, 16)
    pxa[:, :NP, 16] = 1.0
    pxa[:, 500, 17] = 1.0
    # stage layout: [core, p, wv, i, pp, g, j]
    px8 = np.ascontiguousarray(
        pxa.reshape(NC, WAVES, GPW, 2, 2, 128, 18).transpose(0, 5, 1, 4, 3, 2, 6)
    ).reshape(NC, 128, WAVES * 2 * 2 * GPW * 18).astype(float8_e4m3)

    # ---- v-side adjacency (with vb1 carrier row) ----
    vsrc, vdst = vei[0], vei[1]
    vdeg = 1.0 + np.bincount(vdst, minlength=B * NV).astype(f32)
    vdinv = (1.0 / np.sqrt(vdeg)).astype(f32)
    AvT = np.zeros((B, NVP, NVP), f32)
    wv_ = (vdinv[vsrc] * vdinv[vdst]).astype(f32)
    np.add.at(AvT, (vdst // NV, vsrc % NV, vdst % NV), wv_)
    arv = np.arange(B * NV)
    AvT[arv // NV, arv % NV, arv % NV] += vdinv * vdinv
    AvT[:, 50, :NV] = 1.0                           # vb1 carrier
    avt_pair = np.zeros((B // 2, 128, 128), f32)
    avt_pair[:, :NVP, :NVP] = AvT[0::2]
    avt_pair[:, NVP:, NVP:] = AvT[1::2]
    avt = np.ascontiguousarray(
        avt_pair.reshape(NC, 16, 128, 128).transpose(0, 2, 1, 3)
    ).reshape(NC, 128, 2048).astype(bfloat16)

    vxa = np.zeros((B, NVP, 18), f32)
    vxa[:, :NV, :16] = vx.reshape(B, NV, 16)
    vxa[:, :NV, 16] = 1.0
    vxa[:, 50, 17] = 1.0
    vxt = np.ascontiguousarray(
        vxa.reshape(NC, 16, 128, 18).transpose(0, 2, 1, 3)
    ).reshape(NC, 128, 16 * 18).astype(bfloat16)
    vxTa = np.ascontiguousarray(
        vxa[:, :, :17].reshape(NC, GPC, NVP, 17).transpose(0, 3, 1, 2)
    ).reshape(NC, 17, VN).astype(bfloat16)

    # gexp [8, VN]: per quarter window of 512 cols (64-padded),
    # row q covers graph 8Q+q
    gexp = np.zeros((8, VN), f32)
    for Q in range(4):
        for qq in range(8):
            gl = 8 * Q + qq
            gexp[qq, gl * NVP:(gl + 1) * NVP] = 1.0

    # ---- weights ----
    w01x = np.concatenate(
        [(g["pW0"] @ g["pW1"]) / PSCALE, (g["pb0"] @ g["pW1"])[None] / PSCALE,
         g["pb1"][None] / PSCALE], 0)
    w01v = np.concatenate(
        [g["vW0"] @ g["vW1"], (g["vb0"] @ g["vW1"])[None], g["vb1"][None]], 0)
    w01x4 = np.zeros((128, 128), f32)
    w01v4 = np.zeros((128, 128), f32)
    for a in range(4):
        w01x4[32 * a:32 * a + 18] = w01x
        w01v4[32 * a:32 * a + 18] = w01v

    hW1 = g["hW1"]
    W_a = g["vW2"] @ hW1[0:128]
    w0bv = np.concatenate([g["vW0"], g["vb0"][None]], 0)
    W_b = w0bv @ hW1[128:256]
    W_c = g["vW2"] @ hW1[256:384] / NV
    W_d = g["pW2"] @ hW1[384:512] * 0.5
    hb1c = (g["hb1"] + g["vb2"] @ hW1[0:128] + g["vb2"] @ hW1[256:384]
            + g["pb2"] @ hW1[384:512])
    hw2c = np.ascontiguousarray(
        g["hW2"].reshape(2, 128, 128).transpose(1, 0, 2)).reshape(128, 256)

    shared = {
        "gexp": gexp, "w01x4": w01x4, "w01v4": w01v4,
        "W_a": W_a, "W_b": W_b, "W_c": W_c, "W_d": W_d,
        "hw2c": hw2c, "hw3": g["hW3"],
    }
    fblob = np.zeros((128, 4), f32)
    fblob[:, 0:2] = np.ascontiguousarray(hb1c.reshape(2, 128).T)
    fblob[:, 2] = g["hb2"]
    fblob[0, 3] = g["hb3"][0]

    in_maps = []
    for c in range(NC):
        bblob = np.zeros((128, BCOLS), bfloat16)
        for name, arr in {**shared, "avt": avt[c], "vxt": vxt[c],
                          "vxTa": vxTa[c]}.items():
            P, F, off = BSPEC[name]
            bblob[0:P, off:off + F] = arr.astype(bfloat16)
        in_maps.append({
            "pa": pa8[c],
            "px": px8[c],
            "bblob": bblob,
            "fblob": fblob,
        })
    return in_maps


def _ensure_ntff_hook():
    """Provide antenv.axon_hooks if the image lacks it, so trace=True works."""
    try:
        from antenv.axon_hooks import get_axon_ntff_profile_hook  # noqa: F401
        return
    except ImportError:
        pass
    try:
        import sys
        import types
        import antenv
        from trn_agent_boot.trn_boot import _ntff_profile_via_ctypes

        hook = _ntff_profile_via_ctypes("/opt/axon/libaxon_pjrt.so")
        mod = types.ModuleType("antenv.axon_hooks")
        mod._hook = hook
        mod.get_axon_ntff_profile_hook = lambda: mod._hook
        mod.set_axon_ntff_profile_hook = lambda h: setattr(mod, "_hook", h)
        sys.modules["antenv.axon_hooks"] = mod
        antenv.axon_hooks = mod
    except Exception:
        pass


def kernel(**inputs):
    global _nc_cache, LAST_RESULTS
    from concourse.bass_utils import run_bass_kernel_spmd

    in_maps = _host_prep(inputs)
    if _nc_cache is None:
        _nc_cache = _build_nc()
    trace = os.environ.get("KERNEL_TRACE", "0") == "1"
    if trace:
        _ensure_ntff_hook()
    res = run_bass_kernel_spmd(_nc_cache, in_maps, core_ids=list(range(NC)),
                               trace=trace)
    LAST_RESULTS = res
    outs = [res.results[c]["out"].reshape(GPC, NV) for c in range(NC)]
    return np.concatenate(outs, 0).astype(np.float32)


# revision 24
# speedup vs baseline: 1.0662x; 1.0662x over previous
"""Trainium2 Bass kernel for nn_ActorNetwork (GNN message passing), 8 NeuronCores.

Strategy
--------
Data-parallel over the 256 graphs: core c owns graphs [32c, 32c+32).

Algebraic restructure (validated vs reference to ~7e-7 rel err in f32,
~5.7e-3 with fp8/bf16 quantization):
  * GCNConv aggregation is a dense per-graph matmul with the block-diagonal
    normalized adjacency built on host from the edge list.
  * p-encoder only feeds its mean-pooled graph embedding forward:
      p_graph = pW2^T (sum_n relu(q[:, n])) + pb2,  q = W01^T (x_aug^T A_c)
    The relu-sum uses sum relu(x) = (sum x + sum |x|)/2: the linear part
    comes free as an extra adjacency column (row-sums), the |x| part is one
    DVE abs-reduce per graph straight out of PSUM - no relu materialization.
  * The layer-1 bias term pb1*c'[n] rides through the adjacency matmul as an
    extra source row (row 500 = c'), paired with an indicator column in x_aug.
  * mm1 (x_aug^T A_c) runs as fp8 DoubleRow matmuls (K=256/instr, 0.5
    cycles/row).  All 4 graphs of a wave share one K=2048 accumulation with a
    block-diagonal stationary (graph g at stationary cols [32g, 32g+18)),
    so the whole wave lands in ONE [128, 504] PSUM bank -> one cast drains 4
    graphs.  The stationary zero-padding is materialized on-chip (gpsimd
    memset once + small expansion copies), not shipped over HBM.
  * v-encoder: vb1 bias rides through A_v via a fake pad-node row; per-pair
    PSUM tiles are stacked 4-to-a-bank (col tile positions / free slots with
    a shared K-base per bank) so drains are [128, 512].
  * Head folds: h2/h0 are never materialized; hW1 blocks are pre-multiplied
    into W_a = vW2@hW1[0:128] (consumes av1), W_b = [vW0;vb0]@hW1[128:256]
    (consumes vxTa), W_c = vW2@hW1[256:384]/NV (consumes sum(av1)),
    W_d = pW2@hW1[384:512]*0.5 (consumes Z); all constant bias terms fold
    into hb1c.  The head runs in 4 quarters pipelined with the p-waves.
"""

import os
import numpy as np
from ml_dtypes import bfloat16, float8_e4m3

B, NP, NV, E = 256, 500, 50, 128
NC = 8
GPC = B // NC          # 32 graphs per core
NVP = 64               # padded v nodes per graph
VN = GPC * NVP         # 2048 padded v nodes per core
WAVES = 8
GPW = GPC // WAVES     # 4 graphs per wave
NCOLP = 504            # 500 dst cols + 1 rowsum col + 3 pad
PSCALE = 128.0         # fp8 scale folded into w01x


def _mk_bspec():
    bf = {}
    off = 0
    for name, P, F in [("avt", 128, 2048), ("vxt", 128, 16 * 18),
                       ("w01v4", 128, 128), ("w01x4", 128, 128),
                       ("W_a", 128, 256), ("W_c", 128, 256),
                       ("W_d", 128, 256), ("hw2c", 128, 256),
                       ("hw3", 128, 1),
                       ("vxTa", 17, VN), ("W_b", 17, 256),
                       ("gexp", 8, VN)]:
        bf[name] = (P, F, off)
        off += F
    return bf, off


BSPEC, BCOLS = _mk_bspec()

LAST_RESULTS = None
_nc_cache = None


def _build_nc():
    import concourse.bass as bass
    import concourse.bacc as bacc
    import concourse.mybir as mybir
    from concourse.tile import TileContext

    dt = mybir.dt
    f32, bf16 = dt.float32, dt.bfloat16
    AF = mybir.ActivationFunctionType
    AX = mybir.AxisListType
    OP = mybir.AluOpType
    DR = mybir.MatmulPerfMode.DoubleRow

    nc = bacc.Bacc("TRN2", target_bir_lowering=False, debug=False)

    pa_d = nc.declare_dram_parameter("pa", [WAVES, 128, 8 * 2 * NCOLP],
                                     dt.float8e4, isOutput=False)
    px_d = nc.declare_dram_parameter("px", [128, WAVES * 2 * 2 * GPW * 18],
                                     dt.float8e4, isOutput=False)
    bb_d = nc.declare_dram_parameter("bblob", [128, BCOLS], bf16, isOutput=False)
    fb_d = nc.declare_dram_parameter("fblob", [128, 4], f32, isOutput=False)
    out_d = nc.declare_dram_parameter("out", [1, GPC * NV], f32, isOutput=True)

    with TileContext(nc) as tc:
        with (
            tc.tile_pool(name="const", bufs=1) as cp,
            tc.tile_pool(name="pa", bufs=4) as pap,
            tc.tile_pool(name="ya", bufs=3) as yap,
            tc.tile_pool(name="big", bufs=1) as bp,
            tc.tile_pool(name="psA", bufs=2, space="PSUM") as psA,
            tc.tile_pool(name="psB", bufs=3, space="PSUM") as psB,
            tc.tile_pool(name="psC", bufs=2, space="PSUM") as psC,
            tc.tile_pool(name="psD", bufs=1, space="PSUM") as psD,
        ):
            bb = cp.tile([128, BCOLS], bf16, tag="bb", name="bb")
            for name in BSPEC:
                P, F, off = BSPEC[name]
                nc.sync.dma_start(out=bb[0:P, off:off + F],
                                  in_=bb_d[0:P, off:off + F])
            fb = cp.tile([128, 4], f32, tag="fb", name="fb")
            nc.sync.dma_start(out=fb[:], in_=fb_d[:])
            pxs = cp.tile([128, WAVES * 2 * 2 * GPW * 18], dt.float8e4,
                          tag="pxs", name="pxs")
            nc.sync.dma_start(out=pxs[:], in_=px_d[:])

            def bslc(name):
                P, F, off = BSPEC[name]
                return bb[0:P, off:off + F]

            avt = bslc("avt")
            vxt = bslc("vxt")
            vxTa = bslc("vxTa")
            gexp = bslc("gexp")
            w01x4 = bslc("w01x4")
            w01v4 = bslc("w01v4")
            W_a = bslc("W_a")
            W_b = bslc("W_b")
            W_c = bslc("W_c")
            W_d = bslc("W_d")
            hw2c = bslc("hw2c")
            hw3 = bslc("hw3")

            # persistent SBUF tiles
            px_ts = [cp.tile([128, 2176], dt.float8e4, tag=f"pxb{i}",
                             name=f"pxb{i}") for i in range(4)]
            for i in range(4):
                nc.scalar.memzero(px_ts[i][:])
            warm = bp.tile([1, 4], f32, tag="warm")
            nc.scalar.activation(out=warm[:], in_=fb[0:1, 0:4],
                                 func=AF.Lrelu, alpha=0.01)
            nc.scalar.activation(out=warm[:], in_=warm[:], func=AF.Abs)
            yav_t = bp.tile([128, 512], bf16, tag="yav")
            h1_t = bp.tile([128, VN], bf16, tag="h1")
            av1_t = bp.tile([128, VN], bf16, tag="av1")
            Sav_f = bp.tile([128, GPC], f32, tag="Savf")
            Sav_t = bp.tile([128, GPC], bf16, tag="Sav")
            Za = bp.tile([128, GPC], f32, tag="Za")    # DVE-written
            Zb = bp.tile([128, GPC], f32, tag="Zb")    # Act-written
            Zs = bp.tile([128, GPC], f32, tag="Zs")    # Act-written
            Zs2 = bp.tile([128, GPC], f32, tag="Zs2")  # DVE-written
            Z_t = bp.tile([128, GPC], bf16, tag="Z")
            scr = bp.tile([128, 504], bf16, tag="scr")
            gg_t = bp.tile([8, 256], bf16, tag="gg")
            xh_ts = [bp.tile([128, GPC * NV], bf16, tag=f"xh{b}", name=f"xh{b}")
                     for b in range(2)]
            hm_t = bp.tile([128, GPC * NV], bf16, tag="hm")
            ob = bp.tile([1, GPC * NV], f32, tag="ob")

            # ---------------- v encoder (emitted interleaved with waves) --
            def v_yav():
                yav_ps = psC.tile([128, 512], f32, tag="mC", name="yav_ps")
                for j in range(16):
                    nc.tensor.matmul(
                        out=yav_ps[32 * (j % 4):32 * (j % 4) + 18,
                                   128 * (j // 4):128 * (j // 4) + 128],
                        lhsT=vxt[:, 18 * j:18 * j + 18],
                        rhs=avt[:, 128 * j:128 * j + 128],
                        start=True, stop=True,
                        tile_position=(0, 32 * (j % 4)),
                    )
                nc.vector.tensor_copy(out=yav_t[:], in_=yav_ps[:])

            def v_qv():
                h1v = h1_t[:].rearrange("p (c f q) -> p c f q", c=4, f=4)
                for r in range(4):
                    a = 32 * r
                    qv_ps = psC.tile([128, 512], f32, tag="mC", name="qv_ps")
                    for c in range(4):
                        nc.tensor.matmul(
                            out=qv_ps[:, 128 * c:128 * c + 128],
                            lhsT=yav_t[a:a + 18, 128 * c:128 * c + 128],
                            rhs=w01v4[a:a + 18, :],
                            start=True, stop=True,
                            tile_position=(a, 0),
                        )
                    nc.scalar.activation(out=h1v[:, :, r, :], in_=qv_ps[:],
                                         func=AF.Relu)

            def v_av1():
                for jj in range(4):
                    av_ps = psC.tile([128, 512], f32, tag="mC", name="av_ps")
                    for j2 in range(4):
                        j = jj * 4 + j2
                        nc.tensor.matmul(
                            out=av_ps[:, 128 * j2:128 * j2 + 128],
                            lhsT=h1_t[:, 128 * j:128 * j + 128],
                            rhs=avt[:, 128 * j:128 * j + 128],
                            start=True, stop=True,
                        )
                    nc.vector.tensor_copy(
                        out=av1_t[:, 512 * jj:512 * jj + 512], in_=av_ps[:])

            def v_sav():
                nc.vector.tensor_reduce(
                    out=Sav_f[:],
                    in_=av1_t[:].rearrange("p (g n) -> p g n",
                                           n=NVP)[:, :, 0:NV],
                    axis=AX.X, op=OP.add,
                )
                nc.vector.tensor_copy(out=Sav_t[:], in_=Sav_f[:])

            # ---------------- p waves + pipelined head quarters ----------
            def mm1(wv):
                pa_t = pap.tile([128, 8 * 2 * NCOLP], dt.float8e4,
                                tag="pa", name="pa_t")
                H = 4 * 2 * NCOLP
                nc.gpsimd.dma_start(out=pa_t[:, 0:H], in_=pa_d[wv][:, 0:H])
                nc.gpsimd.dma_start(out=pa_t[:, H:2 * H],
                                    in_=pa_d[wv][:, H:2 * H])
                # expand this wave's x_aug blocks into the zero-padded
                # block-diagonal stationary buffer (4 strided copies)
                pxb = px_ts[wv % 4]
                for i in range(2):
                    for pp in range(2):
                        soff = ((wv * 2 + i) * 2 + pp) * GPW * 18
                        nc.gpsimd.tensor_copy(
                            out=pxb[:, 1024 * i + 512 * pp:
                                    1024 * i + 512 * pp + 640].rearrange(
                                "p (g r) -> p g r", g=4)[:, :, 0:18],
                            in_=pxs[:, soff:soff + GPW * 18].rearrange(
                                "p (g j) -> p g j", g=4),
                        )
                pxv = pxb[:, 0:2048].rearrange("p (i q) -> p i q", i=2)
                yps = psA.tile([128, NCOLP], f32, tag="mA", name="yps")
                for kp in range(8):
                    nc.tensor.matmul(
                        out=yps[:],
                        lhsT=pxv[:, :, 128 * kp:128 * kp + 128],
                        rhs=pa_t[:, 2 * NCOLP * kp:2 * NCOLP * (kp + 1)]
                            .rearrange("p (i c) -> p i c", i=2),
                        start=(kp == 0), stop=(kp == 7),
                        perf_mode=DR,
                    )
                ya_t = yap.tile([128, NCOLP], bf16, tag="ya", name="ya_t")
                nc.vector.tensor_copy(out=ya_t[:], in_=yps[:])
                return ya_t

            def mm2(wv, ya_t):
                for gj in range(GPW):
                    g = wv * GPW + gj
                    a = 32 * gj
                    qps = psB.tile([128, NCOLP], f32, tag="mB", name="qps")
                    nc.tensor.matmul(
                        out=qps[:],
                        lhsT=w01x4[a:a + 18, :],
                        rhs=ya_t[a:a + 18, :],
                        start=True, stop=True,
                        tile_position=(a, 0),
                    )
                    if gj == 3:
                        nc.scalar.activation(out=scr[:, 0:500],
                                             in_=qps[:, 0:500], func=AF.Abs,
                                             accum_out=Zb[:, g:g + 1])
                        nc.vector.tensor_copy(out=Zs2[:, g:g + 1],
                                              in_=qps[:, 500:501])
                    else:
                        nc.vector.tensor_reduce(
                            out=Za[:, g:g + 1], in_=qps[:, 0:500],
                            axis=AX.X, op=OP.add, apply_absolute_value=True,
                        )
                        nc.scalar.activation(out=Zs[:, g:g + 1],
                                             in_=qps[:, 500:501], func=AF.Copy)

            def head_pre(Q):
                Sv = slice(512 * Q, 512 * Q + 512)
                banks = []
                for blk in range(2):
                    bs = slice(128 * blk, 128 * blk + 128)
                    xh_ps = psC.tile([128, 512], f32, tag="mC", name="xh_ps")
                    nc.tensor.matmul(out=xh_ps[:], lhsT=W_a[:, bs],
                                     rhs=av1_t[:, Sv], start=True, stop=False)
                    nc.tensor.matmul(out=xh_ps[:], lhsT=W_b[:, bs],
                                     rhs=vxTa[:, Sv], start=False, stop=False)
                    banks.append(xh_ps)
                return banks

            def head_quarter(Q, banks):
                gs = slice(8 * Q, 8 * Q + 8)
                S = slice(400 * Q, 400 * Q + 400)
                Ztv = Z_t[:].rearrange("p (w j) -> p w j", j=4)
                Wv = slice(2 * Q, 2 * Q + 2)
                nc.vector.tensor_tensor(
                    out=Ztv[:, Wv, 0:3],
                    in0=Za[:].rearrange("p (w j) -> p w j", j=4)[:, Wv, 0:3],
                    in1=Zs[:].rearrange("p (w j) -> p w j", j=4)[:, Wv, 0:3],
                    op=OP.add)
                nc.vector.tensor_tensor(
                    out=Ztv[:, Wv, 3:4],
                    in0=Zb[:].rearrange("p (w j) -> p w j", j=4)[:, Wv, 3:4],
                    in1=Zs2[:].rearrange("p (w j) -> p w j", j=4)[:, Wv, 3:4],
                    op=OP.add)
                gg_ps = psD.tile([8, 256], f32, tag="mD", name="gg_ps")
                nc.tensor.matmul(out=gg_ps[:], lhsT=Sav_t[:, gs], rhs=W_c[:],
                                 start=True, stop=False)
                nc.tensor.matmul(out=gg_ps[:], lhsT=Z_t[:, gs], rhs=W_d[:],
                                 start=False, stop=True)
                nc.vector.tensor_copy(out=gg_t[:], in_=gg_ps[:])
                Sv = slice(512 * Q, 512 * Q + 512)
                for blk in range(2):
                    bs = slice(128 * blk, 128 * blk + 128)
                    xh_ps = banks[blk]
                    nc.tensor.matmul(out=xh_ps[:], lhsT=gg_t[:, bs],
                                     rhs=gexp[:, Sv],
                                     start=False, stop=True)
                    nc.scalar.activation(
                        out=xh_ts[blk][:, S],
                        in_=xh_ps[:].rearrange("p (g n) -> p g n", n=NVP)[:, :, 0:NV],
                        func=AF.Lrelu,
                        bias=fb[:, blk:blk + 1], alpha=0.01)
                hm_ps = psC.tile([128, 400], f32, tag="mC", name="hm_ps")
                nc.tensor.matmul(out=hm_ps[:], lhsT=hw2c[:, 0:128],
                                 rhs=xh_ts[0][:, S], start=True, stop=False)
                nc.tensor.matmul(out=hm_ps[:], lhsT=hw2c[:, 128:256],
                                 rhs=xh_ts[1][:, S], start=False, stop=True)
                nc.scalar.activation(out=hm_t[:, S], in_=hm_ps[:],
                                     func=AF.Lrelu, bias=fb[:, 2:3],
                                     alpha=0.01)
                ob_ps = psD.tile([1, 400], f32, tag="mD", name="ob_ps")
                nc.tensor.matmul(out=ob_ps[:], lhsT=hw3[:], rhs=hm_t[:, S],
                                 start=True, stop=True)
                nc.scalar.activation(out=ob[:, S], in_=ob_ps[:],
                                     func=AF.Identity, bias=fb[0:1, 3:4])
                nc.sync.dma_start(out=out_d[:, S], in_=ob[:, S])

            # software pipeline: emit mm2(wv-1) before mm1(wv) so the PE has
            # ready work queued while wave wv's pa DMA is still streaming;
            # head quarters fire as their graphs complete.
            v_yav()
            ya0 = mm1(0)
            v_qv()
            ya1 = mm1(1)
            v_av1()
            mm2(0, ya0)
            v_sav()
            hbanks = head_pre(0)
            ya2 = mm1(2)
            mm2(1, ya1)
            head_quarter(0, hbanks)
            yas = {2: ya2}
            for wv in range(3, WAVES):
                mm2(wv - 1, yas.pop(wv - 1))
                if (wv - 1) % 2 == 0:
                    hbanks = head_pre((wv - 1) // 2)
                else:
                    head_quarter((wv - 1) // 2, hbanks)
                yas[wv] = mm1(wv)
            mm2(WAVES - 1, yas.pop(WAVES - 1))
            head_quarter(3, hbanks)

    nc.compile()
    return nc


def _host_prep(inp):
    f32 = np.float32
    px = np.asarray(inp["p_x"], f32)
    vx = np.asarray(inp["v_x"], f32)
    pei = np.asarray(inp["p_edge_index"]).astype(np.int64)
    vei = np.asarray(inp["v_edge_index"]).astype(np.int64)
    g = {k: np.asarray(inp[k], f32) for k in
         ("pW0", "pb0", "pW1", "pb1", "pW2", "pb2",
          "vW0", "vb0", "vW1", "vb1", "vW2", "vb2",
          "hW1", "hb1", "hW2", "hb2", "hW3", "hb3")}

    # ---- p-side adjacency ----
    psrc, pdst = pei[0], pei[1]
    pdeg = 1.0 + np.bincount(pdst, minlength=B * NP).astype(f32)
    pdinv = (1.0 / np.sqrt(pdeg)).astype(f32)
    csum = pdinv * np.bincount(psrc, weights=pdinv[pdst],
                               minlength=B * NP).astype(f32)
    cp = (csum + pdinv * pdinv) / NP
    pa = np.zeros((B, 512, NCOLP), f32)
    w = (pdinv[psrc] * pdinv[pdst] * cp[pdst]).astype(f32)
    np.add.at(pa, (pdst // NP, psrc % NP, pdst % NP), w)
    ar = np.arange(B * NP)
    pa[ar // NP, ar % NP, ar % NP] += pdinv * pdinv * cp
    pa[:, 500, :500] = cp.reshape(B, NP)            # pb1 carrier row
    pa[:, :, 500] = pa[:, :, :500].sum(axis=2)      # rowsum col (Zs)
    # [B, 512, 504] -> [core, wave, p, kp'=pp*4+g, i, c]
    pa8 = (np.ascontiguousarray(
        (pa * PSCALE).reshape(NC, WAVES, GPW, 2, 2, 128, NCOLP)
        .transpose(0, 1, 5, 3, 2, 4, 6)
    ).reshape(NC, WAVES, 128, 8 * 2 * NCOLP)).astype(float8_e4m3)

    pxa = np.zeros((B, 512, 18), f32)
    pxa[:, :NP, :16] = px.reshape(B, NP, 16)
    pxa[:, :NP, 16] = 1.0
    pxa[:, 500, 17] = 1.0
    # stage layout: [core, p, wv, i, pp, g, j]
    px8 = np.ascontiguousarray(
        pxa.reshape(NC, WAVES, GPW, 2, 2, 128, 18).transpose(0, 5, 1, 4, 3, 2, 6)
    ).reshape(NC, 128, WAVES * 2 * 2 * GPW * 18).astype(float8_e4m3)

    # ---- v-side adjacency (with vb1 carrier row) ----
    vsrc, vdst = vei[0], vei[1]
    vdeg = 1.0 + np.bincount(vdst, minlength=B * NV).astype(f32)
    vdinv = (1.0 / np.sqrt(vdeg)).astype(f32)
    AvT = np.zeros((B, NVP, NVP), f32)
    wv_ = (vdinv[vsrc] * vdinv[vdst]).astype(f32)
    np.add.at(AvT, (vdst // NV, vsrc % NV, vdst % NV), wv_)
    arv = np.arange(B * NV)
    AvT[arv // NV, arv % NV, arv % NV] += vdinv * vdinv
    AvT[:, 50, :NV] = 1.0                           # vb1 carrier
    avt_pair = np.zeros((B // 2, 128, 128), f32)
    avt_pair[:, :NVP, :NVP] = AvT[0::2]
    avt_pair[:, NVP:, NVP:] = AvT[1::2]
    avt = np.ascontiguousarray(
        avt_pair.reshape(NC, 16, 128, 128).transpose(0, 2, 1, 3)
    ).reshape(NC, 128, 2048).astype(bfloat16)

    vxa = np.zeros((B, NVP, 18), f32)
    vxa[:, :NV, :16] = vx.reshape(B, NV, 16)
    vxa[:, :NV, 16] = 1.0
    vxa[:, 50, 17] = 1.0
    vxt = np.ascontiguousarray(
        vxa.reshape(NC, 16, 128, 18).transpose(0, 2, 1, 3)
    ).reshape(NC, 128, 16 * 18).astype(bfloat16)
    vxTa = np.ascontiguousarray(
        vxa[:, :, :17].reshape(NC, GPC, NVP, 17).transpose(0, 3, 1, 2)
    ).reshape(NC, 17, VN).astype(bfloat16)

    # gexp [8, VN]: per quarter window of 512 cols (64-padded),
    # row q covers graph 8Q+q
    gexp = np.zeros((8, VN), f32)
    for Q in range(4):
        for qq in range(8):
            gl = 8 * Q + qq
            gexp[qq, gl * NVP:(gl + 1) * NVP] = 1.0

    # ---- weights ----
    w01x = np.concatenate(
        [(g["pW0"] @ g["pW1"]) / PSCALE, (g["pb0"] @ g["pW1"])[None] / PSCALE,
         g["pb1"][None] / PSCALE], 0)
    w01v = np.concatenate(
        [g["vW0"] @ g["vW1"], (g["vb0"] @ g["vW1"])[None], g["vb1"][None]], 0)
    w01x4 = np.zeros((128, 128), f32)
    w01v4 = np.zeros((128, 128), f32)
    for a in range(4):
        w01x4[32 * a:32 * a + 18] = w01x
        w01v4[32 * a:32 * a + 18] = w01v

    hW1 = g["hW1"]
    W_a = g["vW2"] @ hW1[0:128]
    w0bv = np.concatenate([g["vW0"], g["vb0"][None]], 0)
    W_b = w0bv @ hW1[128:256]
    W_c = g["vW2"] @ hW1[256:384] / NV
    W_d = g["pW2"] @ hW1[384:512] * 0.5
    hb1c = (g["hb1"] + g["vb2"] @ hW1[0:128] + g["vb2"] @ hW1[256:384]
            + g["pb2"] @ hW1[384:512])
    hw2c = np.ascontiguousarray(
        g["hW2"].reshape(2, 128, 128).transpose(1, 0, 2)).reshape(128, 256)

    shared = {
        "gexp": gexp, "w01x4": w01x4, "w01v4": w01v4,
        "W_a": W_a, "W_b": W_b, "W_c": W_c, "W_d": W_d,
        "hw2c": hw2c, "hw3": g["hW3"],
    }
    fblob = np.zeros((128, 4), f32)
    fblob[:, 0:2] = np.ascontiguousarray(hb1c.reshape(2, 128).T)
    fblob[:, 2] = g["hb2"]
    fblob[0, 3] = g["hb3"][0]

    in_maps = []
    for c in range(NC):
        bblob = np.zeros((128, BCOLS), bfloat16)
        for name, arr in {**shared, "avt": avt[c], "vxt": vxt[c],
                          "vxTa": vxTa[c]}.items():
            P, F, off = BSPEC[name]
            bblob[0:P, off:off + F] = arr.astype(bfloat16)
        in_maps.append({
            "pa": pa8[c],
            "px": px8[c],
            "bblob": bblob,
            "fblob": fblob,
        })
    return in_maps


def _ensure_ntff_hook():
    """Provide antenv.axon_hooks if the image lacks it, so trace=True works."""
    try:
        from antenv.axon_hooks import get_axon_ntff_profile_hook  # noqa: F401
        return
    except ImportError:
        pass
    try:
        import sys
        import types
        import antenv
        from trn_agent_boot.trn_boot import _ntff_profile_via_ctypes

        hook = _ntff_profile_via_ctypes("/opt/axon/libaxon_pjrt.so")
        mod = types.ModuleType("antenv.axon_hooks")
        mod._hook = hook
        mod.get_axon_ntff_profile_hook = lambda: mod._hook
        mod.set_axon_ntff_profile_hook = lambda h: setattr(mod, "_hook", h)
        sys.modules["antenv.axon_hooks"] = mod
        antenv.axon_hooks = mod
    except Exception:
        pass


def kernel(**inputs):
    global _nc_cache, LAST_RESULTS
    from concourse.bass_utils import run_bass_kernel_spmd

    in_maps = _host_prep(inputs)
    if _nc_cache is None:
        _nc_cache = _build_nc()
    trace = os.environ.get("KERNEL_TRACE", "0") == "1"
    if trace:
        _ensure_ntff_hook()
    res = run_bass_kernel_spmd(_nc_cache, in_maps, core_ids=list(range(NC)),
                               trace=trace)
    LAST_RESULTS = res
    outs = [res.results[c]["out"].reshape(GPC, NV) for c in range(NC)]
    return np.concatenate(outs, 0).astype(np.float32)
